# revision 31
# baseline (speedup 1.0000x reference)
"""Energy Transformer descent kernel for 8 Trainium2 NeuronCores.

Problem: 12 steps of gradient descent on
  E(x) = -(1/beta) sum logsumexp(beta q k^T) - 0.5 sum relu(g xi^T)^2,
  g = LayerNorm(x; gamma, delta), q = g Wq_h, k = g Wk_h.

Sharding: data-parallel over batch B=4 -> core pairs (2b, 2b+1); within a
pair, core j takes attention heads j*6..j*6+5 and Hopfield memories
xi[j*1536:(j+1)*1536].  Both energy terms contribute additively to dE/dx
and LayerNorm-backward is linear in the upstream gradient, so each core
computes a partial dx and a pairwise AllReduce produces the full step.

Host-side preprocessing folds gamma and the attention scale into the
weights (delta must be zero, which the problem guarantees):
  Wq' = sqrt(beta) diag(gamma) Wq      (forward projections)
  WqT' = (1/sqrt(beta)) (diag(gamma) Wq)^T   (gradient projections)
  xi' = xi diag(gamma)
so the kernel never touches gamma/delta and computes true gradients.

Attention avoids all P-matrix transposes: both S = q k^T (row chunks)
and S^T = k q^T are computed directly on the PE from qT/kT, exp'd on the
scalar engine (unnormalized), and the softmax 1/Z is folded in as a
per-partition scale of q (for dk^T) and a broadcast-row multiply of the
dq^T PSUM (for dq^T).  dg is accumulated *untransposed* ([token, d]) in
8 PSUM banks using the transposed intermediates (dqT/dkT/relu(h)^T) as
stationary operands, so no gradient transposes are needed at the tail
and LayerNorm-backward reads straight from PSUM.
"""

import numpy as np

import concourse.bass as bass
import concourse.tile as tile
from concourse import bacc, mybir

STEPS = 12
ALPHA = 0.125
EPS = 1e-5
B, N, D, H, HD, M = 4, 512, 768, 12, 64, 3072
P = 128
NT = N // P  # 4 row chunks
DT = D // P  # 6 embed chunks
HL = H // 2  # heads per core
EW = HL * HD  # 384 local head width
ET = EW // P  # 3 stacked head-pair chunks
ML = M // 2  # memories per core
MT = ML // P  # 12 memory chunks
F32 = mybir.dt.float32
F32R = mybir.dt.float32r
BF16 = mybir.dt.bfloat16
AF = mybir.ActivationFunctionType
OP = mybir.AluOpType

REPLICA_GROUPS = [[0, 1], [2, 3], [4, 5], [6, 7]]

# d-segments for the untransposed dg accumulation (PSUM bank = 512 f32)
DSEGS = ((0, 512), (512, 256))


def f_(ap):
    return ap.bitcast(F32)


def build_kernel(steps=STEPS, with_ar=True, debug_phase=99, debug_dump=False):
    nc = bacc.Bacc("TRN2", target_bir_lowering=False, debug=False, num_devices=8)

    x_in = nc.declare_dram_parameter("x", [N, D], F32, isOutput=False)
    wq_d = nc.declare_dram_parameter("wq", [D, EW], BF16, isOutput=False)
    wk_d = nc.declare_dram_parameter("wk", [D, EW], BF16, isOutput=False)
    wqt_d = nc.declare_dram_parameter("wqt", [EW, D], BF16, isOutput=False)
    wkt_d = nc.declare_dram_parameter("wkt", [EW, D], BF16, isOutput=False)
    xi_d = nc.declare_dram_parameter("xi", [ML, D], BF16, isOutput=False)
    xit_d = nc.declare_dram_parameter("xit", [D, ML], BF16, isOutput=False)
    x_out = nc.declare_dram_parameter("x_out", [N, D], F32, isOutput=True)

    with tile.TileContext(nc) as tc:
        import contextlib

        with contextlib.ExitStack() as ctx:
            consts = ctx.enter_context(tc.tile_pool(name="consts", bufs=1))
            work = ctx.enter_context(tc.tile_pool(name="work", bufs=1))
            attp = ctx.enter_context(tc.tile_pool(name="attp", bufs=2))
            stats = ctx.enter_context(tc.tile_pool(name="stats", bufs=4))
            stream = ctx.enter_context(tc.tile_pool(name="stream", bufs=4))
            rtp = ctx.enter_context(tc.tile_pool(name="rtp", bufs=1))
            xip = ctx.enter_context(tc.tile_pool(name="xip", bufs=1))
            scr = ctx.enter_context(tc.tile_pool(name="scr", bufs=2))
            drp = ctx.enter_context(tc.tile_pool(name="drp", bufs=2, space="DRAM"))

            # ---- resident tensors ----
            wq_sb = consts.tile([P, DT, EW], BF16)
            nc.sync.dma_start(out=wq_sb[:], in_=wq_d.rearrange("(dt p) e -> p dt e", p=P))
            wk_sb = consts.tile([P, DT, EW], BF16)
            nc.sync.dma_start(out=wk_sb[:], in_=wk_d.rearrange("(dt p) e -> p dt e", p=P))
            wqt_sb = consts.tile([P, ET, D], BF16)
            nc.sync.dma_start(out=wqt_sb[:], in_=wqt_d.rearrange("(et p) d -> p et d", p=P))
            wkt_sb = consts.tile([P, ET, D], BF16)
            nc.sync.dma_start(out=wkt_sb[:], in_=wkt_d.rearrange("(et p) d -> p et d", p=P))
            x_sb = consts.tile([P, NT, D], F32)
            nc.sync.dma_start(out=x_sb[:], in_=x_in.rearrange("(nt p) d -> p nt d", p=P))
            xi_sb = consts.tile([P, MT, D], BF16)
            nc.sync.dma_start(out=xi_sb[:], in_=xi_d.rearrange("(mt p) d -> p mt d", p=P))
            xit_sb = consts.tile([P, DT, ML], BF16)
            nc.sync.dma_start(out=xit_sb[:], in_=xit_d.rearrange("(dt p) m -> p dt m", p=P))

            from concourse.masks import make_identity

            ident_f = consts.tile([P, P], F32)
            make_identity(nc, ident_f[:])
            ident = consts.tile([P, P], F32R)
            nc.vector.tensor_copy(out=ident[:], in_=ident_f[:])
            ident_b = consts.tile([P, P], BF16)
            nc.vector.tensor_copy(out=ident_b[:], in_=ident_f[:])
            eps_t = consts.tile([P, 1], F32)
            nc.vector.memset(eps_t[:], EPS)

            F16 = mybir.dt.float16
            peer_prev = None
            for step in range(steps):
                pswa_ctx = tc.tile_pool(name="pswa", bufs=5, space="PSUM")
                pswA = pswa_ctx.__enter__()
                pswb_ctx = tc.tile_pool(name="pswb", bufs=3, space="PSUM")
                pswB = pswb_ctx.__enter__()

                # ======== x update (deferred from previous step's AllGather)
                # + LayerNorm forward, chunk-pipelined with the gT transposes
                # so the PE starts as soon as chunk 0 is ready ========
                xhatb = work.tile([P, NT, D], BF16, tag="xhatb")
                rstd = stats.tile([P, NT], F32, tag="rstd")
                gT = work.tile([P, DT, N], BF16, tag="gT")
                gtp = [
                    pswB.tile([P, 2, 512], BF16, tag="pswb", name=f"gtp{i}")
                    for i in range(ET)
                ]

                dxs = work.tile([P, NT, D], BF16, tag="dxs")
                for nt in range(NT):
                    if peer_prev is not None:
                        nc.gpsimd.tensor_tensor(
                            out=dxs[:, nt, :], in0=peer_prev[:, 0, nt, :],
                            in1=peer_prev[:, 1, nt, :], op=OP.add,
                        )
                        nc.vector.scalar_tensor_tensor(
                            out=x_sb[:, nt, :], in0=dxs[:, nt, :], scalar=ALPHA,
                            in1=x_sb[:, nt, :], op0=OP.mult, op1=OP.add,
                        )
                    xt = x_sb[:, nt, :]
                    st = stats.tile([P, 3, 6], F32, tag="bnst")
                    xg = xt.rearrange("p (g s) -> p g s", s=256)
                    for gs in range(3):
                        nc.vector.bn_stats(out=st[:, gs, :], in_=xg[:, gs, :])
                    mv = stats.tile([P, 2], F32, tag="mv")
                    nc.vector.bn_aggr(out=mv[:], in_=st[:])
                    rr = rstd[:, nt : nt + 1]
                    nc.scalar.activation(out=rr, in_=mv[:, 1:2], func=AF.Sqrt, bias=eps_t[:], scale=1.0)
                    nc.vector.reciprocal(out=rr, in_=rr)
                    nmu = stats.tile([P, 1], F32, tag="nmu")
                    nc.vector.scalar_tensor_tensor(
                        out=nmu[:], in0=mv[:, 0:1], scalar=-1.0, in1=rr, op0=OP.mult, op1=OP.mult,
                    )
                    nc.vector.tensor_scalar(
                        out=xhatb[:, nt, :], in0=xt, scalar1=rr, scalar2=nmu[:],
                        op0=OP.mult, op1=OP.add,
                    )
                    # gT transposes for this chunk (columns nt of every dt)
                    for dt in range(DT):
                        nc.tensor.transpose(
                            gtp[dt // 2][:, dt % 2, nt * P : (nt + 1) * P],
                            xhatb[:, nt, dt * P : (dt + 1) * P], ident_b[:],
                        )
                for dp in range(ET):
                    nc.vector.tensor_copy(out=gT[:, 2 * dp : 2 * dp + 2, :], in_=gtp[dp][:])

                # ======== projections ========
                q = work.tile([P, NT, EW], BF16, tag="q")
                k = work.tile([P, NT, EW], BF16, tag="k")
                for nt in range(NT):
                    ppq = pswA.tile([P, 512], F32, tag="pswa")
                    ppk = pswA.tile([P, 512], F32, tag="pswa")
                    for dt in range(DT):
                        lh = gT[:, dt, nt * P : (nt + 1) * P]
                        nc.tensor.matmul(ppq[:, :EW], lh, wq_sb[:, dt, :], start=(dt == 0), stop=(dt == DT - 1))
                        nc.tensor.matmul(ppk[:, :EW], lh, wk_sb[:, dt, :], start=(dt == 0), stop=(dt == DT - 1))
                    nc.vector.tensor_copy(out=q[:, nt, :], in_=ppq[:, :EW])
                    nc.vector.tensor_copy(out=k[:, nt, :], in_=ppk[:, :EW])
                qT = work.tile([P, ET, N], BF16, tag="qT")
                kT = work.tile([P, ET, N], BF16, tag="kT")
                for dst, srct in ((qT, q), (kT, k)):
                    for et in range(ET):
                        pp = pswB.tile([P, 512], BF16, tag="pswb")
                        for nt in range(NT):
                            nc.tensor.transpose(
                                pp[:, nt * P : (nt + 1) * P],
                                srct[:, nt, et * P : (et + 1) * P], ident_b[:],
                            )
                        nc.vector.tensor_copy(out=dst[:, et, :], in_=pp[:])

                # ======== attention heads fused with Hopfield phase 1 ========
                # Per head: S/ST chunk matmuls + exps, then two Hopfield
                # h-chains (PE filler while the scalar engine runs the exps),
                # then dq/dk for the previous head (whose E/ET are done).
                dqTst = work.tile([P, ET, N], BF16, tag="dqTst")
                dkTst = work.tile([P, ET, N], BF16, tag="dkTst")

                hctx = {}
                rts = []

                def emit_sst(h):
                    et, eo = h // 2, (h % 2) * HD
                    E = attp.tile([P, NT, N], BF16, tag="E")
                    ETt = attp.tile([P, NT, N], BF16, tag="ETt")
                    Z4 = attp.tile([P, NT], F32, tag="Z4")
                    Zi4 = attp.tile([P, NT], F32, tag="Zi4")
                    zrow = attp.tile([1, N], F32, tag="zrow")
                    ZinvB = attp.tile([HD, N], F32, tag="ZinvB")
                    qs = attp.tile([P, NT, HD], BF16, tag="qs")
                    # S = q k^T row chunks -> exp -> E (unnormalized) + Z sums
                    for nt in range(NT):
                        ps = pswA.tile([P, 512], F32, tag="pswa")
                        nc.tensor.matmul(
                            ps[:], qT[eo : eo + HD, et, nt * P : (nt + 1) * P],
                            kT[eo : eo + HD, et, :], start=True, stop=True,
                        )
                        nc.scalar.activation(out=E[:, nt, :], in_=ps[:], func=AF.Exp)
                        nc.vector.tensor_reduce(
                            Z4[:, nt : nt + 1], E[:, nt, :], mybir.AxisListType.X, OP.add,
                        )
                    # S^T = k q^T -> ET (unnormalized)
                    for jt in range(NT):
                        ps = pswA.tile([P, 512], F32, tag="pswa")
                        nc.tensor.matmul(
                            ps[:], kT[eo : eo + HD, et, jt * P : (jt + 1) * P],
                            qT[eo : eo + HD, et, :], start=True, stop=True,
                        )
                        nc.scalar.activation(out=ETt[:, jt, :], in_=ps[:], func=AF.Exp)
                    hctx[h] = (E, ETt, Z4, Zi4, zrow, ZinvB, qs)

                def emit_hop_pair(mt0):
                    hps = []
                    for mt in (mt0, mt0 + 1):
                        hps.append(pswB.tile([P, 512], F32, tag="pswb", name=f"hp{mt}"))
                    for dt in range(DT):
                        for c in range(2):
                            mt = mt0 + c
                            nc.tensor.matmul(
                                hps[c][:], xit_sb[:, dt, mt * P : (mt + 1) * P], gT[:, dt, :],
                                start=(dt == 0), stop=(dt == DT - 1),
                            )
                    for c, mt in enumerate((mt0, mt0 + 1)):
                        RT = rtp.tile([P, N], BF16, tag=f"RT{mt}")
                        nc.vector.tensor_scalar_max(out=RT[:], in0=hps[c][:], scalar1=0.0)
                        rts.append(RT)

                def emit_dqdk(h):
                    et, eo = h // 2, (h % 2) * HD
                    E, ETt, Z4, Zi4, zrow, ZinvB, qs = hctx.pop(h)
                    # Zinv column form (q scale) and broadcast row form (for
                    # the dqT free-dim scale); deferred one head so the tiny
                    # transposes never wait on the scalar exp backlog
                    nc.vector.reciprocal(out=Zi4[:], in_=Z4[:])
                    for nt in range(NT):
                        nc.vector.tensor_scalar_mul(
                            out=qs[:, nt, :], in0=q[:, nt, h * HD : (h + 1) * HD],
                            scalar1=Zi4[:, nt : nt + 1],
                        )
                    Zr = attp.tile([P, NT], F32R, tag="Zr")
                    nc.vector.tensor_copy(out=Zr[:], in_=Zi4[:])
                    ztp = pswB.tile([P, 512], F32, tag="pswb")
                    for c in range(NT):
                        nc.tensor.transpose(
                            ztp[:1, c * P : (c + 1) * P].bitcast(F32R), Zr[:, c : c + 1], ident[:],
                        )
                    nc.vector.tensor_copy(out=zrow[:1, :], in_=ztp[:1, :])
                    nc.gpsimd.partition_broadcast(ZinvB[:], zrow[:1, :], channels=HD)
                    # dkT_h = sum_i q'_ie E_ij
                    pk = pswA.tile([P, 512], F32, tag="pswa")
                    for nt in range(NT):
                        nc.tensor.matmul(
                            pk[:HD, :], qs[:, nt, :], E[:, nt, :],
                            start=(nt == 0), stop=(nt == NT - 1),
                        )
                    nc.vector.tensor_copy(out=dkTst[eo : eo + HD, et, :], in_=pk[:HD, :])
                    # dqT_h = (sum_j k_je ET_ji) * Zinv_i
                    pq = pswA.tile([P, 512], F32, tag="pswa")
                    for jt in range(NT):
                        nc.tensor.matmul(
                            pq[:HD, :], k[:, jt, h * HD : (h + 1) * HD], ETt[:, jt, :],
                            start=(jt == 0), stop=(jt == NT - 1),
                        )
                    nc.vector.tensor_tensor(
                        out=dqTst[eo : eo + HD, et, :], in0=pq[:HD, :], in1=ZinvB[:], op=OP.mult,
                    )

                for h in range(HL):
                    emit_sst(h)
                    emit_hop_pair(2 * h)
                    if h > 0:
                        emit_dqdk(h - 1)

                emit_dqdk(HL - 1)
                pswb_ctx.__exit__(None, None, None)
                pswa_ctx.__exit__(None, None, None)

                # ======== phase 2: dg accumulation, untransposed [token, d] ========
                psdg_ctx = tc.tile_pool(name="psdg", bufs=1, space="PSUM")
                psdg = psdg_ctx.__enter__()
                dx = work.tile([P, NT, D], F32, tag="dx")
                dxb = work.tile([P, NT, D], BF16, tag="dxb")
                m1s = stats.tile([P, 2, NT], F32, tag="m1s")
                for nt in range(NT):
                    pds = [
                        psdg.tile([P, 512], F32, tag=f"pd{nt}{si}", name=f"pd{nt}{si}")
                        for si in range(len(DSEGS))
                    ]
                    for si, (dlo, dw) in enumerate(DSEGS):
                        for et in range(ET):
                            for d_t, w_t in ((dqTst, wqt_sb), (dkTst, wkt_sb)):
                                nc.tensor.matmul(
                                    pds[si][:, :dw], d_t[:, et, nt * P : (nt + 1) * P],
                                    w_t[:, et, dlo : dlo + dw],
                                    start=(et == 0 and d_t is dqTst), stop=False,
                                )
                    for si, (dlo, dw) in enumerate(DSEGS):
                        for mt in range(MT):
                            nc.tensor.matmul(
                                pds[si][:, :dw], rts[mt][:, nt * P : (nt + 1) * P],
                                xi_sb[:, mt, dlo : dlo + dw],
                                start=False, stop=(mt == MT - 1),
                            )
                    nc.vector.scalar_tensor_tensor(
                        out=dx[:, nt, 0:512], in0=pds[0][:], scalar=0.0, in1=xhatb[:, nt, 0:512],
                        op0=OP.bypass, op1=OP.bypass, accum_out=m1s[:, 0, nt : nt + 1],
                    )
                    nc.vector.scalar_tensor_tensor(
                        out=dx[:, nt, 512:768], in0=pds[1][:, :256], scalar=0.0, in1=xhatb[:, nt, 512:768],
                        op0=OP.bypass, op1=OP.bypass, accum_out=m1s[:, 1, nt : nt + 1],
                    )
                    # LayerNorm backward for this chunk (dx holds dg) -- kept
                    # inside the chunk loop so it overlaps later chunks' PE
                    # chains and feeds the first AllGather half early
                    rr = rstd[:, nt : nt + 1]
                    m1 = stats.tile([P, 1], F32, tag="m1")
                    nc.vector.tensor_tensor(out=m1[:], in0=m1s[:, 0, nt : nt + 1], in1=m1s[:, 1, nt : nt + 1], op=OP.add)
                    prodA = scr.tile([P, D], F32, tag="prodA")
                    u2 = stats.tile([P, 1], F32, tag="u2")
                    nc.vector.scalar_tensor_tensor(
                        out=prodA[:], in0=dx[:, nt, :], scalar=1.0, in1=xhatb[:, nt, :],
                        op0=OP.mult, op1=OP.mult, accum_out=u2[:],
                    )
                    c1 = stats.tile([P, 1], F32, tag="c1")
                    nc.vector.scalar_tensor_tensor(
                        out=c1[:], in0=m1[:], scalar=1.0 / D, in1=rr, op0=OP.mult, op1=OP.mult,
                    )
                    c2 = stats.tile([P, 1], F32, tag="c2")
                    nc.vector.scalar_tensor_tensor(
                        out=c2[:], in0=u2[:], scalar=-1.0 / D, in1=rr, op0=OP.mult, op1=OP.mult,
                    )
                    lnv = scr.tile([P, D], F32, tag="lnv")
                    nc.vector.tensor_scalar(
                        out=lnv[:], in0=dx[:, nt, :], scalar1=rr, scalar2=c1[:],
                        op0=OP.mult, op1=OP.subtract,
                    )
                    nc.vector.scalar_tensor_tensor(
                        out=dxb[:, nt, :], in0=xhatb[:, nt, :], scalar=c2[:], in1=lnv[:],
                        op0=OP.mult, op1=OP.add,
                    )
                psdg_ctx.__exit__(None, None, None)

                # ======== pair exchange (AllGather: no slow CC-core reduce;
                # the pair sum is folded into the deferred x update).  Split
                # in two halves: the first overlaps the second half of the
                # dg accumulation / LayerNorm-backward. ========
                if with_ar:
                    peer = work.tile([P, 2, NT, D], BF16, tag="peer")
                    HN = N // 2
                    arouts = []
                    for g in range(2):
                        arin = drp.tile([HN, D], BF16, tag=f"arin{g}", name=f"arin{g}")
                        arout = drp.tile([2 * HN, D], BF16, tag=f"arout{g}", name=f"arout{g}")
                        for c in range(2):
                            nt = 2 * g + c
                            nc.sync.dma_start(out=arin[c * P : (c + 1) * P, :], in_=dxb[:, nt, :])
                        nc.gpsimd.collective_compute(
                            "AllGather", OP.bypass, replica_groups=REPLICA_GROUPS,
                            ins=[arin.opt()], outs=[arout.opt()],
                        )
                        arouts.append(arout)
                    # peer readbacks AFTER both collectives are queued, so the
                    # second collective's inputs are not stuck behind them
                    for g in range(2):
                        for r in range(2):
                            nc.sync.dma_start(
                                out=peer[:, r, 2 * g : 2 * g + 2, :],
                                in_=arouts[g][r * HN : (r + 1) * HN, :].rearrange(
                                    "(c p) d -> p c d", p=P
                                ),
                            )
                    peer_prev = peer
                else:
                    for nt in range(NT):
                        nc.vector.scalar_tensor_tensor(
                            out=x_sb[:, nt, :], in0=dxb[:, nt, :], scalar=ALPHA,
                            in1=x_sb[:, nt, :], op0=OP.mult, op1=OP.add,
                        )

            if peer_prev is not None:
                dxf = work.tile([P, NT, D], BF16, tag="dxs")
                for nt in range(NT):
                    nc.gpsimd.tensor_tensor(
                        out=dxf[:, nt, :], in0=peer_prev[:, 0, nt, :],
                        in1=peer_prev[:, 1, nt, :], op=OP.add,
                    )
                    nc.vector.scalar_tensor_tensor(
                        out=x_sb[:, nt, :], in0=dxf[:, nt, :], scalar=ALPHA,
                        in1=x_sb[:, nt, :], op0=OP.mult, op1=OP.add,
                    )
            for nt in range(NT):
                nc.sync.dma_start(out=x_out[nt * P : (nt + 1) * P, :], in_=x_sb[:, nt, :])

    nc.compile()
    return nc


def _prep_inputs(x, gamma, delta, Wq, Wk, xi):
    """Build the 8 per-core input dicts (host-side sharding + weight folding)."""
    assert np.allclose(delta, 0.0), "kernel requires delta == 0"
    beta_sqrt = np.float32(1.0 / np.sqrt(np.sqrt(np.float32(HD))))
    # sqrt(beta) = (1/sqrt(HD))^(1/2) = HD^(-1/4)
    g = gamma.astype(np.float32)
    in_maps = []
    for c in range(8):
        b, j = c // 2, c % 2
        hs = slice(j * HL, (j + 1) * HL)
        wq_l = (Wq[hs] * g[None, :, None]).transpose(1, 0, 2).reshape(D, EW)
        wk_l = (Wk[hs] * g[None, :, None]).transpose(1, 0, 2).reshape(D, EW)
        wqt_l = (Wq[hs] * g[None, :, None]).transpose(0, 2, 1).reshape(EW, D)
        wkt_l = (Wk[hs] * g[None, :, None]).transpose(0, 2, 1).reshape(EW, D)
        xi_l = xi[j * ML : (j + 1) * ML] * g[None, :]
        import ml_dtypes

        bf = ml_dtypes.bfloat16
        in_maps.append(
            {
                "x": np.ascontiguousarray(x[b]),
                "wq": np.ascontiguousarray(wq_l * beta_sqrt).astype(bf),
                "wk": np.ascontiguousarray(wk_l * beta_sqrt).astype(bf),
                "wqt": np.ascontiguousarray(wqt_l / beta_sqrt).astype(bf),
                "wkt": np.ascontiguousarray(wkt_l / beta_sqrt).astype(bf),
                "xi": np.ascontiguousarray(xi_l).astype(bf),
                "xit": np.ascontiguousarray(xi_l.T).astype(bf),
            }
        )
    return in_maps


_NC_CACHE = {}


def _get_nc(steps=STEPS, with_ar=True):
    key = (steps, with_ar)
    if key not in _NC_CACHE:
        _NC_CACHE[key] = build_kernel(steps, with_ar)
    return _NC_CACHE[key]


def kernel(x, gamma, delta, Wq, Wk, xi):
    from concourse.bass_utils import run_bass_kernel_spmd

    x = np.asarray(x, dtype=np.float32)
    in_maps = _prep_inputs(
        x,
        np.asarray(gamma, np.float32),
        np.asarray(delta, np.float32),
        np.asarray(Wq, np.float32),
        np.asarray(Wk, np.float32),
        np.asarray(xi, np.float32),
    )
    nc = _get_nc()
    res = run_bass_kernel_spmd(nc, in_maps, list(range(8)))
    out = np.stack([res.results[2 * b]["x_out"] for b in range(B)], axis=0)
    return out.astype(np.float32)


# revision 32
# speedup vs baseline: 1.0482x; 1.0482x over previous
"""Energy Transformer descent kernel for 8 Trainium2 NeuronCores.

Problem: 12 steps of gradient descent on
  E(x) = -(1/beta) sum logsumexp(beta q k^T) - 0.5 sum relu(g xi^T)^2,
  g = LayerNorm(x; gamma, delta), q = g Wq_h, k = g Wk_h.

Sharding: data-parallel over batch B=4 -> core pairs (2b, 2b+1); within a
pair, core j takes attention heads j*6..j*6+5 and Hopfield memories
xi[j*1536:(j+1)*1536].  Both energy terms contribute additively to dE/dx
and LayerNorm-backward is linear in the upstream gradient, so each core
computes a partial dx and a pairwise AllReduce produces the full step.

Host-side preprocessing folds gamma and the attention scale into the
weights (delta must be zero, which the problem guarantees):
  Wq' = sqrt(beta) diag(gamma) Wq      (forward projections)
  WqT' = (1/sqrt(beta)) (diag(gamma) Wq)^T   (gradient projections)
  xi' = xi diag(gamma)
so the kernel never touches gamma/delta and computes true gradients.

Attention avoids all P-matrix transposes: both S = q k^T (row chunks)
and S^T = k q^T are computed directly on the PE from qT/kT, exp'd on the
scalar engine (unnormalized), and the softmax 1/Z is folded in as a
per-partition scale of q (for dk^T) and a broadcast-row multiply of the
dq^T PSUM (for dq^T).  dg is accumulated *untransposed* ([token, d]) in
8 PSUM banks using the transposed intermediates (dqT/dkT/relu(h)^T) as
stationary operands, so no gradient transposes are needed at the tail
and LayerNorm-backward reads straight from PSUM.
"""

import numpy as np

import concourse.bass as bass
import concourse.tile as tile
from concourse import bacc, mybir

STEPS = 12
ALPHA = 0.125
EPS = 1e-5
B, N, D, H, HD, M = 4, 512, 768, 12, 64, 3072
P = 128
NT = N // P  # 4 row chunks
DT = D // P  # 6 embed chunks
HL = H // 2  # heads per core
EW = HL * HD  # 384 local head width
ET = EW // P  # 3 stacked head-pair chunks
ML = M // 2  # memories per core
MT = ML // P  # 12 memory chunks
F32 = mybir.dt.float32
F32R = mybir.dt.float32r
BF16 = mybir.dt.bfloat16
AF = mybir.ActivationFunctionType
OP = mybir.AluOpType

REPLICA_GROUPS = [[0, 1], [2, 3], [4, 5], [6, 7]]

# d-segments for the untransposed dg accumulation (PSUM bank = 512 f32)
DSEGS = ((0, 512), (512, 256))


def f_(ap):
    return ap.bitcast(F32)


def build_kernel(steps=STEPS, with_ar=True, debug_phase=99, debug_dump=False):
    nc = bacc.Bacc("TRN2", target_bir_lowering=False, debug=False, num_devices=8)

    x_in = nc.declare_dram_parameter("x", [N, D], F32, isOutput=False)
    wq_d = nc.declare_dram_parameter("wq", [D, EW], BF16, isOutput=False)
    wk_d = nc.declare_dram_parameter("wk", [D, EW], BF16, isOutput=False)
    wqt_d = nc.declare_dram_parameter("wqt", [EW, D], BF16, isOutput=False)
    wkt_d = nc.declare_dram_parameter("wkt", [EW, D], BF16, isOutput=False)
    xi_d = nc.declare_dram_parameter("xi", [ML, D], BF16, isOutput=False)
    xit_d = nc.declare_dram_parameter("xit", [D, ML], BF16, isOutput=False)
    x_out = nc.declare_dram_parameter("x_out", [N, D], F32, isOutput=True)

    with tile.TileContext(nc) as tc:
        import contextlib

        with contextlib.ExitStack() as ctx:
            consts = ctx.enter_context(tc.tile_pool(name="consts", bufs=1))
            work = ctx.enter_context(tc.tile_pool(name="work", bufs=1))
            attp = ctx.enter_context(tc.tile_pool(name="attp", bufs=2))
            stats = ctx.enter_context(tc.tile_pool(name="stats", bufs=4))
            stream = ctx.enter_context(tc.tile_pool(name="stream", bufs=4))
            rtp = ctx.enter_context(tc.tile_pool(name="rtp", bufs=1))
            xip = ctx.enter_context(tc.tile_pool(name="xip", bufs=1))
            scr = ctx.enter_context(tc.tile_pool(name="scr", bufs=2))
            drp = ctx.enter_context(tc.tile_pool(name="drp", bufs=2, space="DRAM"))

            # ---- resident tensors ----
            wq_sb = consts.tile([P, DT, EW], BF16)
            nc.sync.dma_start(out=wq_sb[:], in_=wq_d.rearrange("(dt p) e -> p dt e", p=P))
            wk_sb = consts.tile([P, DT, EW], BF16)
            nc.sync.dma_start(out=wk_sb[:], in_=wk_d.rearrange("(dt p) e -> p dt e", p=P))
            wqt_sb = consts.tile([P, ET, D], BF16)
            nc.sync.dma_start(out=wqt_sb[:], in_=wqt_d.rearrange("(et p) d -> p et d", p=P))
            wkt_sb = consts.tile([P, ET, D], BF16)
            nc.sync.dma_start(out=wkt_sb[:], in_=wkt_d.rearrange("(et p) d -> p et d", p=P))
            x_sb = consts.tile([P, NT, D], F32)
            nc.sync.dma_start(out=x_sb[:], in_=x_in.rearrange("(nt p) d -> p nt d", p=P))
            xi_sb = consts.tile([P, MT, D], BF16)
            nc.sync.dma_start(out=xi_sb[:], in_=xi_d.rearrange("(mt p) d -> p mt d", p=P))
            xit_sb = consts.tile([P, DT, ML], BF16)
            nc.sync.dma_start(out=xit_sb[:], in_=xit_d.rearrange("(dt p) m -> p dt m", p=P))

            from concourse.masks import make_identity

            ident_f = consts.tile([P, P], F32)
            make_identity(nc, ident_f[:])
            ident = consts.tile([P, P], F32R)
            nc.vector.tensor_copy(out=ident[:], in_=ident_f[:])
            ident_b = consts.tile([P, P], BF16)
            nc.vector.tensor_copy(out=ident_b[:], in_=ident_f[:])
            eps_t = consts.tile([P, 1], F32)
            nc.vector.memset(eps_t[:], EPS)

            F16 = mybir.dt.float16
            peer_prev = None
            for step in range(steps):
                pswa_ctx = tc.tile_pool(name="pswa", bufs=5, space="PSUM")
                pswA = pswa_ctx.__enter__()
                pswb_ctx = tc.tile_pool(name="pswb", bufs=3, space="PSUM")
                pswB = pswb_ctx.__enter__()

                # ======== x update (deferred from previous step's AllGather)
                # + LayerNorm forward, chunk-pipelined with the gT transposes
                # so the PE starts as soon as chunk 0 is ready ========
                xhatb = work.tile([P, NT, D], BF16, tag="xhatb")
                rstd = stats.tile([P, NT], F32, tag="rstd")
                gT = work.tile([P, DT, N], BF16, tag="gT")
                gtp = [
                    pswB.tile([P, 2, 512], BF16, tag="pswb", name=f"gtp{i}")
                    for i in range(ET)
                ]

                for nt in range(NT):
                    if peer_prev is not None:
                        for r in range(2):
                            nc.vector.scalar_tensor_tensor(
                                out=x_sb[:, nt, :], in0=peer_prev[:, r, nt, :], scalar=ALPHA,
                                in1=x_sb[:, nt, :], op0=OP.mult, op1=OP.add,
                            )
                    xt = x_sb[:, nt, :]
                    st = stats.tile([P, 3, 6], F32, tag="bnst")
                    xg = xt.rearrange("p (g s) -> p g s", s=256)
                    for gs in range(3):
                        nc.vector.bn_stats(out=st[:, gs, :], in_=xg[:, gs, :])
                    mv = stats.tile([P, 2], F32, tag="mv")
                    nc.vector.bn_aggr(out=mv[:], in_=st[:])
                    rr = rstd[:, nt : nt + 1]
                    nc.scalar.activation(out=rr, in_=mv[:, 1:2], func=AF.Sqrt, bias=eps_t[:], scale=1.0)
                    nc.vector.reciprocal(out=rr, in_=rr)
                    nmu = stats.tile([P, 1], F32, tag="nmu")
                    nc.vector.scalar_tensor_tensor(
                        out=nmu[:], in0=mv[:, 0:1], scalar=-1.0, in1=rr, op0=OP.mult, op1=OP.mult,
                    )
                    nc.vector.tensor_scalar(
                        out=xhatb[:, nt, :], in0=xt, scalar1=rr, scalar2=nmu[:],
                        op0=OP.mult, op1=OP.add,
                    )
                    # gT transposes for this chunk (columns nt of every dt)
                    for dt in range(DT):
                        nc.tensor.transpose(
                            gtp[dt // 2][:, dt % 2, nt * P : (nt + 1) * P],
                            xhatb[:, nt, dt * P : (dt + 1) * P], ident_b[:],
                        )
                for dp in range(ET):
                    nc.vector.tensor_copy(out=gT[:, 2 * dp : 2 * dp + 2, :], in_=gtp[dp][:])

                # ======== projections ========
                q = work.tile([P, NT, EW], BF16, tag="q")
                k = work.tile([P, NT, EW], BF16, tag="k")
                for nt in range(NT):
                    ppq = pswA.tile([P, 512], F32, tag="pswa")
                    ppk = pswA.tile([P, 512], F32, tag="pswa")
                    for dt in range(DT):
                        lh = gT[:, dt, nt * P : (nt + 1) * P]
                        nc.tensor.matmul(ppq[:, :EW], lh, wq_sb[:, dt, :], start=(dt == 0), stop=(dt == DT - 1))
                        nc.tensor.matmul(ppk[:, :EW], lh, wk_sb[:, dt, :], start=(dt == 0), stop=(dt == DT - 1))
                    nc.vector.tensor_copy(out=q[:, nt, :], in_=ppq[:, :EW])
                    nc.vector.tensor_copy(out=k[:, nt, :], in_=ppk[:, :EW])
                qT = work.tile([P, ET, N], BF16, tag="qT")
                kT = work.tile([P, ET, N], BF16, tag="kT")
                for dst, srct in ((qT, q), (kT, k)):
                    for et in range(ET):
                        pp = pswB.tile([P, 512], BF16, tag="pswb")
                        for nt in range(NT):
                            nc.tensor.transpose(
                                pp[:, nt * P : (nt + 1) * P],
                                srct[:, nt, et * P : (et + 1) * P], ident_b[:],
                            )
                        nc.vector.tensor_copy(out=dst[:, et, :], in_=pp[:])

                # ======== attention heads fused with Hopfield phase 1 ========
                # Per head: S/ST chunk matmuls + exps, then two Hopfield
                # h-chains (PE filler while the scalar engine runs the exps),
                # then dq/dk for the previous head (whose E/ET are done).
                dqTst = work.tile([P, ET, N], BF16, tag="dqTst")
                dkTst = work.tile([P, ET, N], BF16, tag="dkTst")

                hctx = {}
                rts = []

                def emit_sst(h):
                    et, eo = h // 2, (h % 2) * HD
                    E = attp.tile([P, NT, N], BF16, tag="E")
                    ETt = attp.tile([P, NT, N], BF16, tag="ETt")
                    Z4 = attp.tile([P, NT], F32, tag="Z4")
                    Zi4 = attp.tile([P, NT], F32, tag="Zi4")
                    zrow = attp.tile([1, N], F32, tag="zrow")
                    ZinvB = attp.tile([HD, N], F32, tag="ZinvB")
                    qs = attp.tile([P, NT, HD], BF16, tag="qs")
                    # S = q k^T row chunks -> exp -> E (unnormalized) + Z sums
                    for nt in range(NT):
                        ps = pswA.tile([P, 512], F32, tag="pswa")
                        nc.tensor.matmul(
                            ps[:], qT[eo : eo + HD, et, nt * P : (nt + 1) * P],
                            kT[eo : eo + HD, et, :], start=True, stop=True,
                        )
                        nc.scalar.activation(out=E[:, nt, :], in_=ps[:], func=AF.Exp)
                        nc.vector.tensor_reduce(
                            Z4[:, nt : nt + 1], E[:, nt, :], mybir.AxisListType.X, OP.add,
                        )
                    # S^T = k q^T -> ET (unnormalized)
                    for jt in range(NT):
                        ps = pswA.tile([P, 512], F32, tag="pswa")
                        nc.tensor.matmul(
                            ps[:], kT[eo : eo + HD, et, jt * P : (jt + 1) * P],
                            qT[eo : eo + HD, et, :], start=True, stop=True,
                        )
                        nc.scalar.activation(out=ETt[:, jt, :], in_=ps[:], func=AF.Exp)
                    hctx[h] = (E, ETt, Z4, Zi4, zrow, ZinvB, qs)

                def emit_hop_pair(mt0):
                    hps = []
                    for mt in (mt0, mt0 + 1):
                        hps.append(pswB.tile([P, 512], F32, tag="pswb", name=f"hp{mt}"))
                    for dt in range(DT):
                        for c in range(2):
                            mt = mt0 + c
                            nc.tensor.matmul(
                                hps[c][:], xit_sb[:, dt, mt * P : (mt + 1) * P], gT[:, dt, :],
                                start=(dt == 0), stop=(dt == DT - 1),
                            )
                    for c, mt in enumerate((mt0, mt0 + 1)):
                        RT = rtp.tile([P, N], BF16, tag=f"RT{mt}")
                        nc.vector.tensor_scalar_max(out=RT[:], in0=hps[c][:], scalar1=0.0)
                        rts.append(RT)

                def emit_dqdk(h):
                    et, eo = h // 2, (h % 2) * HD
                    E, ETt, Z4, Zi4, zrow, ZinvB, qs = hctx.pop(h)
                    # Zinv column form (q scale) and broadcast row form (for
                    # the dqT free-dim scale); deferred one head so the tiny
                    # transposes never wait on the scalar exp backlog
                    nc.vector.reciprocal(out=Zi4[:], in_=Z4[:])
                    for nt in range(NT):
                        nc.vector.tensor_scalar_mul(
                            out=qs[:, nt, :], in0=q[:, nt, h * HD : (h + 1) * HD],
                            scalar1=Zi4[:, nt : nt + 1],
                        )
                    Zr = attp.tile([P, NT], F32R, tag="Zr")
                    nc.vector.tensor_copy(out=Zr[:], in_=Zi4[:])
                    ztp = pswB.tile([P, 512], F32, tag="pswb")
                    for c in range(NT):
                        nc.tensor.transpose(
                            ztp[:1, c * P : (c + 1) * P].bitcast(F32R), Zr[:, c : c + 1], ident[:],
                        )
                    nc.vector.tensor_copy(out=zrow[:1, :], in_=ztp[:1, :])
                    nc.gpsimd.partition_broadcast(ZinvB[:], zrow[:1, :], channels=HD)
                    # dkT_h = sum_i q'_ie E_ij
                    pk = pswA.tile([P, 512], F32, tag="pswa")
                    for nt in range(NT):
                        nc.tensor.matmul(
                            pk[:HD, :], qs[:, nt, :], E[:, nt, :],
                            start=(nt == 0), stop=(nt == NT - 1),
                        )
                    nc.vector.tensor_copy(out=dkTst[eo : eo + HD, et, :], in_=pk[:HD, :])
                    # dqT_h = (sum_j k_je ET_ji) * Zinv_i
                    pq = pswA.tile([P, 512], F32, tag="pswa")
                    for jt in range(NT):
                        nc.tensor.matmul(
                            pq[:HD, :], k[:, jt, h * HD : (h + 1) * HD], ETt[:, jt, :],
                            start=(jt == 0), stop=(jt == NT - 1),
                        )
                    nc.vector.tensor_tensor(
                        out=dqTst[eo : eo + HD, et, :], in0=pq[:HD, :], in1=ZinvB[:], op=OP.mult,
                    )

                for h in range(HL):
                    emit_sst(h)
                    emit_hop_pair(2 * h)
                    if h > 0:
                        emit_dqdk(h - 1)

                emit_dqdk(HL - 1)
                pswb_ctx.__exit__(None, None, None)
                pswa_ctx.__exit__(None, None, None)

                # ======== phase 2: dg accumulation, untransposed [token, d] ========
                psdg_ctx = tc.tile_pool(name="psdg", bufs=1, space="PSUM")
                psdg = psdg_ctx.__enter__()
                dx = work.tile([P, NT, D], F32, tag="dx")
                dxb = work.tile([P, NT, D], BF16, tag="dxb")
                m1s = stats.tile([P, 2, NT], F32, tag="m1s")
                for nt in range(NT):
                    pds = [
                        psdg.tile([P, 512], F32, tag=f"pd{nt}{si}", name=f"pd{nt}{si}")
                        for si in range(len(DSEGS))
                    ]
                    for si, (dlo, dw) in enumerate(DSEGS):
                        for et in range(ET):
                            for d_t, w_t in ((dqTst, wqt_sb), (dkTst, wkt_sb)):
                                nc.tensor.matmul(
                                    pds[si][:, :dw], d_t[:, et, nt * P : (nt + 1) * P],
                                    w_t[:, et, dlo : dlo + dw],
                                    start=(et == 0 and d_t is dqTst), stop=False,
                                )
                    for si, (dlo, dw) in enumerate(DSEGS):
                        for mt in range(MT):
                            nc.tensor.matmul(
                                pds[si][:, :dw], rts[mt][:, nt * P : (nt + 1) * P],
                                xi_sb[:, mt, dlo : dlo + dw],
                                start=False, stop=(mt == MT - 1),
                            )
                    nc.vector.scalar_tensor_tensor(
                        out=dx[:, nt, 0:512], in0=pds[0][:], scalar=0.0, in1=xhatb[:, nt, 0:512],
                        op0=OP.bypass, op1=OP.bypass, accum_out=m1s[:, 0, nt : nt + 1],
                    )
                    nc.vector.scalar_tensor_tensor(
                        out=dx[:, nt, 512:768], in0=pds[1][:, :256], scalar=0.0, in1=xhatb[:, nt, 512:768],
                        op0=OP.bypass, op1=OP.bypass, accum_out=m1s[:, 1, nt : nt + 1],
                    )
                    # LayerNorm backward for this chunk (dx holds dg) -- kept
                    # inside the chunk loop so it overlaps later chunks' PE
                    # chains and feeds the first AllGather half early
                    rr = rstd[:, nt : nt + 1]
                    m1 = stats.tile([P, 1], F32, tag="m1")
                    nc.vector.tensor_tensor(out=m1[:], in0=m1s[:, 0, nt : nt + 1], in1=m1s[:, 1, nt : nt + 1], op=OP.add)
                    prodA = scr.tile([P, D], F32, tag="prodA")
                    u2 = stats.tile([P, 1], F32, tag="u2")
                    nc.vector.scalar_tensor_tensor(
                        out=prodA[:], in0=dx[:, nt, :], scalar=1.0, in1=xhatb[:, nt, :],
                        op0=OP.mult, op1=OP.mult, accum_out=u2[:],
                    )
                    c1 = stats.tile([P, 1], F32, tag="c1")
                    nc.vector.scalar_tensor_tensor(
                        out=c1[:], in0=m1[:], scalar=1.0 / D, in1=rr, op0=OP.mult, op1=OP.mult,
                    )
                    c2 = stats.tile([P, 1], F32, tag="c2")
                    nc.vector.scalar_tensor_tensor(
                        out=c2[:], in0=u2[:], scalar=-1.0 / D, in1=rr, op0=OP.mult, op1=OP.mult,
                    )
                    lnv = scr.tile([P, D], F32, tag="lnv")
                    nc.vector.tensor_scalar(
                        out=lnv[:], in0=dx[:, nt, :], scalar1=rr, scalar2=c1[:],
                        op0=OP.mult, op1=OP.subtract,
                    )
                    nc.vector.scalar_tensor_tensor(
                        out=dxb[:, nt, :], in0=xhatb[:, nt, :], scalar=c2[:], in1=lnv[:],
                        op0=OP.mult, op1=OP.add,
                    )
                psdg_ctx.__exit__(None, None, None)

                # ======== pair exchange (AllGather: no slow CC-core reduce;
                # the pair sum is folded into the deferred x update).  Split
                # in two halves: the first overlaps the second half of the
                # dg accumulation / LayerNorm-backward. ========
                if with_ar:
                    peer = work.tile([P, 2, NT, D], BF16, tag="peer")
                    HN = N // 2
                    arouts = []
                    for g in range(2):
                        arin = drp.tile([HN, D], BF16, tag=f"arin{g}", name=f"arin{g}")
                        arout = drp.tile([2 * HN, D], BF16, tag=f"arout{g}", name=f"arout{g}")
                        for c in range(2):
                            nt = 2 * g + c
                            nc.sync.dma_start(out=arin[c * P : (c + 1) * P, :], in_=dxb[:, nt, :])
                        nc.gpsimd.collective_compute(
                            "AllGather", OP.bypass, replica_groups=REPLICA_GROUPS,
                            ins=[arin.opt()], outs=[arout.opt()],
                        )
                        arouts.append(arout)
                    # peer readbacks AFTER both collectives are queued, so the
                    # second collective's inputs are not stuck behind them
                    for g in range(2):
                        for r in range(2):
                            nc.sync.dma_start(
                                out=peer[:, r, 2 * g : 2 * g + 2, :],
                                in_=arouts[g][r * HN : (r + 1) * HN, :].rearrange(
                                    "(c p) d -> p c d", p=P
                                ),
                            )
                    peer_prev = peer
                else:
                    for nt in range(NT):
                        nc.vector.scalar_tensor_tensor(
                            out=x_sb[:, nt, :], in0=dxb[:, nt, :], scalar=ALPHA,
                            in1=x_sb[:, nt, :], op0=OP.mult, op1=OP.add,
                        )

            if peer_prev is not None:
                for nt in range(NT):
                    for r in range(2):
                        nc.vector.scalar_tensor_tensor(
                            out=x_sb[:, nt, :], in0=peer_prev[:, r, nt, :], scalar=ALPHA,
                            in1=x_sb[:, nt, :], op0=OP.mult, op1=OP.add,
                        )
            for nt in range(NT):
                nc.sync.dma_start(out=x_out[nt * P : (nt + 1) * P, :], in_=x_sb[:, nt, :])

    nc.compile()
    return nc


def _prep_inputs(x, gamma, delta, Wq, Wk, xi):
    """Build the 8 per-core input dicts (host-side sharding + weight folding)."""
    assert np.allclose(delta, 0.0), "kernel requires delta == 0"
    beta_sqrt = np.float32(1.0 / np.sqrt(np.sqrt(np.float32(HD))))
    # sqrt(beta) = (1/sqrt(HD))^(1/2) = HD^(-1/4)
    g = gamma.astype(np.float32)
    in_maps = []
    for c in range(8):
        b, j = c // 2, c % 2
        hs = slice(j * HL, (j + 1) * HL)
        wq_l = (Wq[hs] * g[None, :, None]).transpose(1, 0, 2).reshape(D, EW)
        wk_l = (Wk[hs] * g[None, :, None]).transpose(1, 0, 2).reshape(D, EW)
        wqt_l = (Wq[hs] * g[None, :, None]).transpose(0, 2, 1).reshape(EW, D)
        wkt_l = (Wk[hs] * g[None, :, None]).transpose(0, 2, 1).reshape(EW, D)
        xi_l = xi[j * ML : (j + 1) * ML] * g[None, :]
        import ml_dtypes

        bf = ml_dtypes.bfloat16
        in_maps.append(
            {
                "x": np.ascontiguousarray(x[b]),
                "wq": np.ascontiguousarray(wq_l * beta_sqrt).astype(bf),
                "wk": np.ascontiguousarray(wk_l * beta_sqrt).astype(bf),
                "wqt": np.ascontiguousarray(wqt_l / beta_sqrt).astype(bf),
                "wkt": np.ascontiguousarray(wkt_l / beta_sqrt).astype(bf),
                "xi": np.ascontiguousarray(xi_l).astype(bf),
                "xit": np.ascontiguousarray(xi_l.T).astype(bf),
            }
        )
    return in_maps


_NC_CACHE = {}


def _get_nc(steps=STEPS, with_ar=True):
    key = (steps, with_ar)
    if key not in _NC_CACHE:
        _NC_CACHE[key] = build_kernel(steps, with_ar)
    return _NC_CACHE[key]


def kernel(x, gamma, delta, Wq, Wk, xi):
    from concourse.bass_utils import run_bass_kernel_spmd

    x = np.asarray(x, dtype=np.float32)
    in_maps = _prep_inputs(
        x,
        np.asarray(gamma, np.float32),
        np.asarray(delta, np.float32),
        np.asarray(Wq, np.float32),
        np.asarray(Wk, np.float32),
        np.asarray(xi, np.float32),
    )
    nc = _get_nc()
    res = run_bass_kernel_spmd(nc, in_maps, list(range(8)))
    out = np.stack([res.results[2 * b]["x_out"] for b in range(B)], axis=0)
    return out.astype(np.float32)


# revision 33
# speedup vs baseline: 1.1222x; 1.0706x over previous
"""Energy Transformer descent kernel for 8 Trainium2 NeuronCores.

Problem: 12 steps of gradient descent on
  E(x) = -(1/beta) sum logsumexp(beta q k^T) - 0.5 sum relu(g xi^T)^2,
  g = LayerNorm(x; gamma, delta), q = g Wq_h, k = g Wk_h.

Sharding: data-parallel over batch B=4 -> core pairs (2b, 2b+1); within a
pair, core j takes attention heads j*6..j*6+5 and Hopfield memories
xi[j*1536:(j+1)*1536].  Both energy terms contribute additively to dE/dx
and LayerNorm-backward is linear in the upstream gradient, so each core
computes a partial dx and a pairwise AllReduce produces the full step.

Host-side preprocessing folds gamma and the attention scale into the
weights (delta must be zero, which the problem guarantees):
  Wq' = sqrt(beta) diag(gamma) Wq      (forward projections)
  WqT' = (1/sqrt(beta)) (diag(gamma) Wq)^T   (gradient projections)
  xi' = xi diag(gamma)
so the kernel never touches gamma/delta and computes true gradients.

Attention avoids all P-matrix transposes: both S = q k^T (row chunks)
and S^T = k q^T are computed directly on the PE from qT/kT, exp'd on the
scalar engine (unnormalized), and the softmax 1/Z is folded in as a
per-partition scale of q (for dk^T) and a broadcast-row multiply of the
dq^T PSUM (for dq^T).  dg is accumulated *untransposed* ([token, d]) in
8 PSUM banks using the transposed intermediates (dqT/dkT/relu(h)^T) as
stationary operands, so no gradient transposes are needed at the tail
and LayerNorm-backward reads straight from PSUM.
"""

import numpy as np

import concourse.bass as bass
import concourse.tile as tile
from concourse import bacc, mybir

STEPS = 12
ALPHA = 0.125
EPS = 1e-5
B, N, D, H, HD, M = 4, 512, 768, 12, 64, 3072
P = 128
NT = N // P  # 4 row chunks
DT = D // P  # 6 embed chunks
HL = H // 2  # heads per core
EW = HL * HD  # 384 local head width
ET = EW // P  # 3 stacked head-pair chunks
ML = M // 2  # memories per core
MT = ML // P  # 12 memory chunks
F32 = mybir.dt.float32
F32R = mybir.dt.float32r
BF16 = mybir.dt.bfloat16
AF = mybir.ActivationFunctionType
OP = mybir.AluOpType

REPLICA_GROUPS = [[0, 1], [2, 3], [4, 5], [6, 7]]

# d-segments for the untransposed dg accumulation (PSUM bank = 512 f32)
DSEGS = ((0, 512), (512, 256))


def f_(ap):
    return ap.bitcast(F32)


def build_kernel(steps=STEPS, with_ar=True, debug_phase=99, debug_dump=False):
    nc = bacc.Bacc("TRN2", target_bir_lowering=False, debug=False, num_devices=8)

    x_in = nc.declare_dram_parameter("x", [N, D], F32, isOutput=False)
    wq_d = nc.declare_dram_parameter("wq", [D, EW], BF16, isOutput=False)
    wk_d = nc.declare_dram_parameter("wk", [D, EW], BF16, isOutput=False)
    wqt_d = nc.declare_dram_parameter("wqt", [EW, D], BF16, isOutput=False)
    wkt_d = nc.declare_dram_parameter("wkt", [EW, D], BF16, isOutput=False)
    xi_d = nc.declare_dram_parameter("xi", [ML, D], BF16, isOutput=False)
    xit_d = nc.declare_dram_parameter("xit", [D, ML], BF16, isOutput=False)
    x_out = nc.declare_dram_parameter("x_out", [N, D], F32, isOutput=True)

    with tile.TileContext(nc) as tc:
        import contextlib

        with contextlib.ExitStack() as ctx:
            consts = ctx.enter_context(tc.tile_pool(name="consts", bufs=1))
            work = ctx.enter_context(tc.tile_pool(name="work", bufs=1))
            attp = ctx.enter_context(tc.tile_pool(name="attp", bufs=2))
            stats = ctx.enter_context(tc.tile_pool(name="stats", bufs=4))
            stream = ctx.enter_context(tc.tile_pool(name="stream", bufs=4))
            rtp = ctx.enter_context(tc.tile_pool(name="rtp", bufs=1))
            xip = ctx.enter_context(tc.tile_pool(name="xip", bufs=1))
            scr = ctx.enter_context(tc.tile_pool(name="scr", bufs=2))
            drp = ctx.enter_context(tc.tile_pool(name="drp", bufs=2, space="DRAM"))

            # ---- resident tensors ----
            wq_sb = consts.tile([P, DT, EW], BF16)
            nc.sync.dma_start(out=wq_sb[:], in_=wq_d.rearrange("(dt p) e -> p dt e", p=P))
            wk_sb = consts.tile([P, DT, EW], BF16)
            nc.sync.dma_start(out=wk_sb[:], in_=wk_d.rearrange("(dt p) e -> p dt e", p=P))
            wqt_sb = consts.tile([P, ET, D], BF16)
            nc.sync.dma_start(out=wqt_sb[:], in_=wqt_d.rearrange("(et p) d -> p et d", p=P))
            wkt_sb = consts.tile([P, ET, D], BF16)
            nc.sync.dma_start(out=wkt_sb[:], in_=wkt_d.rearrange("(et p) d -> p et d", p=P))
            x_sb = consts.tile([P, NT, D], F32)
            nc.sync.dma_start(out=x_sb[:], in_=x_in.rearrange("(nt p) d -> p nt d", p=P))
            xi_sb = consts.tile([P, MT, D], BF16)
            nc.sync.dma_start(out=xi_sb[:], in_=xi_d.rearrange("(mt p) d -> p mt d", p=P))
            xit_sb = consts.tile([P, DT, ML], BF16)
            nc.sync.dma_start(out=xit_sb[:], in_=xit_d.rearrange("(dt p) m -> p dt m", p=P))

            from concourse.masks import make_identity

            ident_f = consts.tile([P, P], F32)
            make_identity(nc, ident_f[:])
            ident = consts.tile([P, P], F32R)
            nc.vector.tensor_copy(out=ident[:], in_=ident_f[:])
            ident_b = consts.tile([P, P], BF16)
            nc.vector.tensor_copy(out=ident_b[:], in_=ident_f[:])
            eps_t = consts.tile([P, 1], F32)
            nc.vector.memset(eps_t[:], EPS)

            F16 = mybir.dt.float16
            peer_prev = None
            for step in range(steps):
                pswa_ctx = tc.tile_pool(name="pswa", bufs=5, space="PSUM")
                pswA = pswa_ctx.__enter__()
                pswb_ctx = tc.tile_pool(name="pswb", bufs=3, space="PSUM")
                pswB = pswb_ctx.__enter__()

                # ======== x update (deferred from previous step's AllGather)
                # + LayerNorm forward, chunk-pipelined with the gT transposes
                # so the PE starts as soon as chunk 0 is ready ========
                xhatb = work.tile([P, NT, D], BF16, tag="xhatb")
                rstd = stats.tile([P, NT], F32, tag="rstd")
                gT = work.tile([P, DT, N], BF16, tag="gT")
                gtp = [
                    pswB.tile([P, 2, 512], BF16, tag="pswb", name=f"gtp{i}")
                    for i in range(ET)
                ]

                for nt in range(NT):
                    if peer_prev is not None:
                        for r in range(2):
                            nc.vector.scalar_tensor_tensor(
                                out=x_sb[:, nt, :], in0=peer_prev[:, r, nt, :], scalar=ALPHA,
                                in1=x_sb[:, nt, :], op0=OP.mult, op1=OP.add,
                            )
                    xt = x_sb[:, nt, :]
                    st = stats.tile([P, 3, 6], F32, tag="bnst")
                    xg = xt.rearrange("p (g s) -> p g s", s=256)
                    for gs in range(3):
                        nc.vector.bn_stats(out=st[:, gs, :], in_=xg[:, gs, :])
                    mv = stats.tile([P, 2], F32, tag="mv")
                    nc.vector.bn_aggr(out=mv[:], in_=st[:])
                    rr = rstd[:, nt : nt + 1]
                    nc.scalar.activation(out=rr, in_=mv[:, 1:2], func=AF.Sqrt, bias=eps_t[:], scale=1.0)
                    nc.vector.reciprocal(out=rr, in_=rr)
                    nmu = stats.tile([P, 1], F32, tag="nmu")
                    nc.vector.scalar_tensor_tensor(
                        out=nmu[:], in0=mv[:, 0:1], scalar=-1.0, in1=rr, op0=OP.mult, op1=OP.mult,
                    )
                    nc.vector.tensor_scalar(
                        out=xhatb[:, nt, :], in0=xt, scalar1=rr, scalar2=nmu[:],
                        op0=OP.mult, op1=OP.add,
                    )
                    # gT transposes for this chunk (columns nt of every dt)
                    for dt in range(DT):
                        nc.tensor.transpose(
                            gtp[dt // 2][:, dt % 2, nt * P : (nt + 1) * P],
                            xhatb[:, nt, dt * P : (dt + 1) * P], ident_b[:],
                        )
                for dp in range(ET):
                    nc.vector.tensor_copy(out=gT[:, 2 * dp : 2 * dp + 2, :], in_=gtp[dp][:])

                # ======== projections ========
                q = work.tile([P, NT, EW], BF16, tag="q")
                k = work.tile([P, NT, EW], BF16, tag="k")
                for nt in range(NT):
                    ppq = pswA.tile([P, 512], F32, tag="pswa")
                    ppk = pswA.tile([P, 512], F32, tag="pswa")
                    for dt in range(DT):
                        lh = gT[:, dt, nt * P : (nt + 1) * P]
                        nc.tensor.matmul(ppq[:, :EW], lh, wq_sb[:, dt, :], start=(dt == 0), stop=(dt == DT - 1))
                        nc.tensor.matmul(ppk[:, :EW], lh, wk_sb[:, dt, :], start=(dt == 0), stop=(dt == DT - 1))
                    nc.vector.tensor_copy(out=q[:, nt, :], in_=ppq[:, :EW])
                    nc.vector.tensor_copy(out=k[:, nt, :], in_=ppk[:, :EW])
                qT = work.tile([P, ET, N], BF16, tag="qT")
                kT = work.tile([P, ET, N], BF16, tag="kT")
                for dst, srct in ((qT, q), (kT, k)):
                    for et in range(ET):
                        pp = pswB.tile([P, 512], BF16, tag="pswb")
                        for nt in range(NT):
                            nc.tensor.transpose(
                                pp[:, nt * P : (nt + 1) * P],
                                srct[:, nt, et * P : (et + 1) * P], ident_b[:],
                            )
                        nc.vector.tensor_copy(out=dst[:, et, :], in_=pp[:])

                # ======== attention heads fused with Hopfield phase 1 ========
                # Per head: S/ST chunk matmuls + exps, then two Hopfield
                # h-chains (PE filler while the scalar engine runs the exps),
                # then dq/dk for the previous head (whose E/ET are done).
                dqTst = work.tile([P, ET, N], BF16, tag="dqTst")
                dkTst = work.tile([P, ET, N], BF16, tag="dkTst")

                hctx = {}
                rts = []

                def emit_sst(h):
                    et, eo = h // 2, (h % 2) * HD
                    E = attp.tile([P, NT, N], BF16, tag="E")
                    ETt = attp.tile([P, NT, N], BF16, tag="ETt")
                    Z4 = attp.tile([P, NT], F32, tag="Z4")
                    Zi4 = attp.tile([P, NT], F32, tag="Zi4")
                    zrow = attp.tile([1, N], F32, tag="zrow")
                    ZinvB = attp.tile([HD, N], F32, tag="ZinvB")
                    qs = attp.tile([P, NT, HD], BF16, tag="qs")
                    # S = q k^T row chunks -> exp -> E (unnormalized) + Z sums
                    for nt in range(NT):
                        ps = pswA.tile([P, 512], F32, tag="pswa")
                        nc.tensor.matmul(
                            ps[:], qT[eo : eo + HD, et, nt * P : (nt + 1) * P],
                            kT[eo : eo + HD, et, :], start=True, stop=True,
                        )
                        nc.scalar.activation(out=E[:, nt, :], in_=ps[:], func=AF.Exp)
                        nc.vector.tensor_reduce(
                            Z4[:, nt : nt + 1], E[:, nt, :], mybir.AxisListType.X, OP.add,
                        )
                    # S^T = k q^T -> ET (unnormalized)
                    for jt in range(NT):
                        ps = pswA.tile([P, 512], F32, tag="pswa")
                        nc.tensor.matmul(
                            ps[:], kT[eo : eo + HD, et, jt * P : (jt + 1) * P],
                            qT[eo : eo + HD, et, :], start=True, stop=True,
                        )
                        nc.scalar.activation(out=ETt[:, jt, :], in_=ps[:], func=AF.Exp)
                    hctx[h] = (E, ETt, Z4, Zi4, zrow, ZinvB, qs)

                def emit_hop_pair(mt0):
                    hps = []
                    for mt in (mt0, mt0 + 1):
                        hps.append(pswB.tile([P, 512], F32, tag="pswb", name=f"hp{mt}"))
                    for dt in range(DT):
                        for c in range(2):
                            mt = mt0 + c
                            nc.tensor.matmul(
                                hps[c][:], xit_sb[:, dt, mt * P : (mt + 1) * P], gT[:, dt, :],
                                start=(dt == 0), stop=(dt == DT - 1),
                            )
                    for c, mt in enumerate((mt0, mt0 + 1)):
                        RT = rtp.tile([P, N], BF16, tag=f"RT{mt}")
                        nc.vector.tensor_scalar_max(out=RT[:], in0=hps[c][:], scalar1=0.0)
                        rts.append(RT)

                def emit_dqdk(h):
                    et, eo = h // 2, (h % 2) * HD
                    E, ETt, Z4, Zi4, zrow, ZinvB, qs = hctx.pop(h)
                    # Zinv column form (q scale) and broadcast row form (for
                    # the dqT free-dim scale); deferred one head so the tiny
                    # transposes never wait on the scalar exp backlog
                    nc.vector.reciprocal(out=Zi4[:], in_=Z4[:])
                    for nt in range(NT):
                        nc.vector.tensor_scalar_mul(
                            out=qs[:, nt, :], in0=q[:, nt, h * HD : (h + 1) * HD],
                            scalar1=Zi4[:, nt : nt + 1],
                        )
                    Zr = attp.tile([P, NT], F32R, tag="Zr")
                    nc.vector.tensor_copy(out=Zr[:], in_=Zi4[:])
                    ztp = pswB.tile([P, 512], F32, tag="pswb")
                    for c in range(NT):
                        nc.tensor.transpose(
                            ztp[:1, c * P : (c + 1) * P].bitcast(F32R), Zr[:, c : c + 1], ident[:],
                        )
                    nc.vector.tensor_copy(out=zrow[:1, :], in_=ztp[:1, :])
                    nc.gpsimd.partition_broadcast(ZinvB[:], zrow[:1, :], channels=HD)
                    # dkT_h = sum_i q'_ie E_ij
                    pk = pswA.tile([P, 512], F32, tag="pswa")
                    for nt in range(NT):
                        nc.tensor.matmul(
                            pk[:HD, :], qs[:, nt, :], E[:, nt, :],
                            start=(nt == 0), stop=(nt == NT - 1),
                        )
                    nc.vector.tensor_copy(out=dkTst[eo : eo + HD, et, :], in_=pk[:HD, :])
                    # dqT_h = (sum_j k_je ET_ji) * Zinv_i
                    pq = pswA.tile([P, 512], F32, tag="pswa")
                    for jt in range(NT):
                        nc.tensor.matmul(
                            pq[:HD, :], k[:, jt, h * HD : (h + 1) * HD], ETt[:, jt, :],
                            start=(jt == 0), stop=(jt == NT - 1),
                        )
                    nc.vector.tensor_tensor(
                        out=dqTst[eo : eo + HD, et, :], in0=pq[:HD, :], in1=ZinvB[:], op=OP.mult,
                    )

                for h in range(HL):
                    emit_sst(h)
                    emit_hop_pair(2 * h)
                    if h > 0:
                        emit_dqdk(h - 1)

                emit_dqdk(HL - 1)
                pswb_ctx.__exit__(None, None, None)
                pswa_ctx.__exit__(None, None, None)

                # ======== phase 2: dg accumulation, untransposed [token, d] ========
                psdg_ctx = tc.tile_pool(name="psdg", bufs=1, space="PSUM")
                psdg = psdg_ctx.__enter__()
                dx = work.tile([P, NT, D], F32, tag="dx")
                dxb = work.tile([P, NT, D], BF16, tag="dxb")
                m1s = stats.tile([P, 2, NT], F32, tag="m1s")
                for nt in range(NT):
                    pds = [
                        psdg.tile([P, 512], F32, tag=f"pd{nt}{si}", name=f"pd{nt}{si}")
                        for si in range(len(DSEGS))
                    ]
                    for si, (dlo, dw) in enumerate(DSEGS):
                        for et in range(ET):
                            for d_t, w_t in ((dqTst, wqt_sb), (dkTst, wkt_sb)):
                                nc.tensor.matmul(
                                    pds[si][:, :dw], d_t[:, et, nt * P : (nt + 1) * P],
                                    w_t[:, et, dlo : dlo + dw],
                                    start=(et == 0 and d_t is dqTst), stop=False,
                                )
                    for si, (dlo, dw) in enumerate(DSEGS):
                        for mt in range(MT):
                            nc.tensor.matmul(
                                pds[si][:, :dw], rts[mt][:, nt * P : (nt + 1) * P],
                                xi_sb[:, mt, dlo : dlo + dw],
                                start=False, stop=(mt == MT - 1),
                            )
                    nc.vector.scalar_tensor_tensor(
                        out=dx[:, nt, 0:512], in0=pds[0][:], scalar=0.0, in1=xhatb[:, nt, 0:512],
                        op0=OP.bypass, op1=OP.bypass, accum_out=m1s[:, 0, nt : nt + 1],
                    )
                    nc.vector.scalar_tensor_tensor(
                        out=dx[:, nt, 512:768], in0=pds[1][:, :256], scalar=0.0, in1=xhatb[:, nt, 512:768],
                        op0=OP.bypass, op1=OP.bypass, accum_out=m1s[:, 1, nt : nt + 1],
                    )
                    # LayerNorm backward for this chunk (dx holds dg) -- kept
                    # inside the chunk loop so it overlaps later chunks' PE
                    # chains and feeds the first AllGather half early
                    rr = rstd[:, nt : nt + 1]
                    m1 = stats.tile([P, 1], F32, tag="m1")
                    nc.vector.tensor_tensor(out=m1[:], in0=m1s[:, 0, nt : nt + 1], in1=m1s[:, 1, nt : nt + 1], op=OP.add)
                    prodA = scr.tile([P, D], F32, tag="prodA")
                    u2 = stats.tile([P, 1], F32, tag="u2")
                    nc.vector.scalar_tensor_tensor(
                        out=prodA[:], in0=dx[:, nt, :], scalar=1.0, in1=xhatb[:, nt, :],
                        op0=OP.mult, op1=OP.mult, accum_out=u2[:],
                    )
                    c1 = stats.tile([P, 1], F32, tag="c1")
                    nc.vector.scalar_tensor_tensor(
                        out=c1[:], in0=m1[:], scalar=1.0 / D, in1=rr, op0=OP.mult, op1=OP.mult,
                    )
                    c2 = stats.tile([P, 1], F32, tag="c2")
                    nc.vector.scalar_tensor_tensor(
                        out=c2[:], in0=u2[:], scalar=-1.0 / D, in1=rr, op0=OP.mult, op1=OP.mult,
                    )
                    lnv = scr.tile([P, D], F32, tag="lnv")
                    nc.vector.tensor_scalar(
                        out=lnv[:], in0=dx[:, nt, :], scalar1=rr, scalar2=c1[:],
                        op0=OP.mult, op1=OP.subtract,
                    )
                    nc.vector.scalar_tensor_tensor(
                        out=dxb[:, nt, :], in0=xhatb[:, nt, :], scalar=c2[:], in1=lnv[:],
                        op0=OP.mult, op1=OP.add,
                    )
                psdg_ctx.__exit__(None, None, None)

                # ======== pair exchange (AllGather: no slow CC-core reduce;
                # the pair sum is folded into the deferred x update).  Split
                # in two halves: the first overlaps the second half of the
                # dg accumulation / LayerNorm-backward. ========
                if with_ar:
                    peer = work.tile([P, 2, NT, D], BF16, tag="peer")
                    HN = N // 2
                    arouts = []
                    for g in range(2):
                        arin = drp.tile([HN, D], BF16, tag=f"arin{g}", name=f"arin{g}")
                        arout = drp.tile([2 * HN, D], BF16, tag=f"arout{g}", name=f"arout{g}")
                        for c in range(2):
                            nt = 2 * g + c
                            nc.sync.dma_start(out=arin[c * P : (c + 1) * P, :], in_=dxb[:, nt, :])
                        nc.gpsimd.collective_compute(
                            "AllGather", OP.bypass, replica_groups=REPLICA_GROUPS,
                            ins=[arin.opt()], outs=[arout.opt()],
                        )
                        arouts.append(arout)
                    # peer readbacks AFTER both collectives are queued, so the
                    # second collective's inputs are not stuck behind them
                    for g in range(2):
                        for r in range(2):
                            for c in range(2):
                                nt = 2 * g + c
                                nc.sync.dma_start(
                                    out=peer[:, r, nt, :],
                                    in_=arouts[g][r * HN + c * P : r * HN + (c + 1) * P, :],
                                )
                    peer_prev = peer
                else:
                    for nt in range(NT):
                        nc.vector.scalar_tensor_tensor(
                            out=x_sb[:, nt, :], in0=dxb[:, nt, :], scalar=ALPHA,
                            in1=x_sb[:, nt, :], op0=OP.mult, op1=OP.add,
                        )

            if peer_prev is not None:
                for nt in range(NT):
                    for r in range(2):
                        nc.vector.scalar_tensor_tensor(
                            out=x_sb[:, nt, :], in0=peer_prev[:, r, nt, :], scalar=ALPHA,
                            in1=x_sb[:, nt, :], op0=OP.mult, op1=OP.add,
                        )
            for nt in range(NT):
                nc.sync.dma_start(out=x_out[nt * P : (nt + 1) * P, :], in_=x_sb[:, nt, :])

    nc.compile()
    return nc


def _prep_inputs(x, gamma, delta, Wq, Wk, xi):
    """Build the 8 per-core input dicts (host-side sharding + weight folding)."""
    assert np.allclose(delta, 0.0), "kernel requires delta == 0"
    beta_sqrt = np.float32(1.0 / np.sqrt(np.sqrt(np.float32(HD))))
    # sqrt(beta) = (1/sqrt(HD))^(1/2) = HD^(-1/4)
    g = gamma.astype(np.float32)
    in_maps = []
    for c in range(8):
        b, j = c // 2, c % 2
        hs = slice(j * HL, (j + 1) * HL)
        wq_l = (Wq[hs] * g[None, :, None]).transpose(1, 0, 2).reshape(D, EW)
        wk_l = (Wk[hs] * g[None, :, None]).transpose(1, 0, 2).reshape(D, EW)
        wqt_l = (Wq[hs] * g[None, :, None]).transpose(0, 2, 1).reshape(EW, D)
        wkt_l = (Wk[hs] * g[None, :, None]).transpose(0, 2, 1).reshape(EW, D)
        xi_l = xi[j * ML : (j + 1) * ML] * g[None, :]
        import ml_dtypes

        bf = ml_dtypes.bfloat16
        in_maps.append(
            {
                "x": np.ascontiguousarray(x[b]),
                "wq": np.ascontiguousarray(wq_l * beta_sqrt).astype(bf),
                "wk": np.ascontiguousarray(wk_l * beta_sqrt).astype(bf),
                "wqt": np.ascontiguousarray(wqt_l / beta_sqrt).astype(bf),
                "wkt": np.ascontiguousarray(wkt_l / beta_sqrt).astype(bf),
                "xi": np.ascontiguousarray(xi_l).astype(bf),
                "xit": np.ascontiguousarray(xi_l.T).astype(bf),
            }
        )
    return in_maps


_NC_CACHE = {}


def _get_nc(steps=STEPS, with_ar=True):
    key = (steps, with_ar)
    if key not in _NC_CACHE:
        _NC_CACHE[key] = build_kernel(steps, with_ar)
    return _NC_CACHE[key]


def kernel(x, gamma, delta, Wq, Wk, xi):
    from concourse.bass_utils import run_bass_kernel_spmd

    x = np.asarray(x, dtype=np.float32)
    in_maps = _prep_inputs(
        x,
        np.asarray(gamma, np.float32),
        np.asarray(delta, np.float32),
        np.asarray(Wq, np.float32),
        np.asarray(Wk, np.float32),
        np.asarray(xi, np.float32),
    )
    nc = _get_nc()
    res = run_bass_kernel_spmd(nc, in_maps, list(range(8)))
    out = np.stack([res.results[2 * b]["x_out"] for b in range(B)], axis=0)
    return out.astype(np.float32)


# revision 34
# speedup vs baseline: 1.1338x; 1.0103x over previous
"""Energy Transformer descent kernel for 8 Trainium2 NeuronCores.

Problem: 12 steps of gradient descent on
  E(x) = -(1/beta) sum logsumexp(beta q k^T) - 0.5 sum relu(g xi^T)^2,
  g = LayerNorm(x; gamma, delta), q = g Wq_h, k = g Wk_h.

Sharding: data-parallel over batch B=4 -> core pairs (2b, 2b+1); within a
pair, core j takes attention heads j*6..j*6+5 and Hopfield memories
xi[j*1536:(j+1)*1536].  Both energy terms contribute additively to dE/dx
and LayerNorm-backward is linear in the upstream gradient, so each core
computes a partial dx and a pairwise AllReduce produces the full step.

Host-side preprocessing folds gamma and the attention scale into the
weights (delta must be zero, which the problem guarantees):
  Wq' = sqrt(beta) diag(gamma) Wq      (forward projections)
  WqT' = (1/sqrt(beta)) (diag(gamma) Wq)^T   (gradient projections)
  xi' = xi diag(gamma)
so the kernel never touches gamma/delta and computes true gradients.

Attention avoids all P-matrix transposes: both S = q k^T (row chunks)
and S^T = k q^T are computed directly on the PE from qT/kT, exp'd on the
scalar engine (unnormalized), and the softmax 1/Z is folded in as a
per-partition scale of q (for dk^T) and a broadcast-row multiply of the
dq^T PSUM (for dq^T).  dg is accumulated *untransposed* ([token, d]) in
8 PSUM banks using the transposed intermediates (dqT/dkT/relu(h)^T) as
stationary operands, so no gradient transposes are needed at the tail
and LayerNorm-backward reads straight from PSUM.
"""

import numpy as np

import concourse.bass as bass
import concourse.tile as tile
from concourse import bacc, mybir

STEPS = 12
ALPHA = 0.125
EPS = 1e-5
B, N, D, H, HD, M = 4, 512, 768, 12, 64, 3072
P = 128
NT = N // P  # 4 row chunks
DT = D // P  # 6 embed chunks
HL = H // 2  # heads per core
EW = HL * HD  # 384 local head width
ET = EW // P  # 3 stacked head-pair chunks
ML = M // 2  # memories per core
MT = ML // P  # 12 memory chunks
F32 = mybir.dt.float32
F32R = mybir.dt.float32r
BF16 = mybir.dt.bfloat16
AF = mybir.ActivationFunctionType
OP = mybir.AluOpType

REPLICA_GROUPS = [[0, 1], [2, 3], [4, 5], [6, 7]]

# d-segments for the untransposed dg accumulation (PSUM bank = 512 f32)
DSEGS = ((0, 512), (512, 256))


def f_(ap):
    return ap.bitcast(F32)


def build_kernel(steps=STEPS, with_ar=True, debug_phase=99, debug_dump=False):
    nc = bacc.Bacc("TRN2", target_bir_lowering=False, debug=False, num_devices=8)

    x_in = nc.declare_dram_parameter("x", [N, D], F32, isOutput=False)
    wq_d = nc.declare_dram_parameter("wq", [D, EW], BF16, isOutput=False)
    wk_d = nc.declare_dram_parameter("wk", [D, EW], BF16, isOutput=False)
    wqt_d = nc.declare_dram_parameter("wqt", [EW, D], BF16, isOutput=False)
    wkt_d = nc.declare_dram_parameter("wkt", [EW, D], BF16, isOutput=False)
    xi_d = nc.declare_dram_parameter("xi", [ML, D], BF16, isOutput=False)
    xit_d = nc.declare_dram_parameter("xit", [D, ML], BF16, isOutput=False)
    x_out = nc.declare_dram_parameter("x_out", [N, D], F32, isOutput=True)

    with tile.TileContext(nc) as tc:
        import contextlib

        with contextlib.ExitStack() as ctx:
            consts = ctx.enter_context(tc.tile_pool(name="consts", bufs=1))
            work = ctx.enter_context(tc.tile_pool(name="work", bufs=1))
            attp = ctx.enter_context(tc.tile_pool(name="attp", bufs=2))
            stats = ctx.enter_context(tc.tile_pool(name="stats", bufs=4))
            stream = ctx.enter_context(tc.tile_pool(name="stream", bufs=4))
            rtp = ctx.enter_context(tc.tile_pool(name="rtp", bufs=1))
            xip = ctx.enter_context(tc.tile_pool(name="xip", bufs=1))
            scr = ctx.enter_context(tc.tile_pool(name="scr", bufs=2))
            drp = ctx.enter_context(tc.tile_pool(name="drp", bufs=2, space="DRAM"))

            # ---- resident tensors ----
            wq_sb = consts.tile([P, DT, EW], BF16)
            nc.sync.dma_start(out=wq_sb[:], in_=wq_d.rearrange("(dt p) e -> p dt e", p=P))
            wk_sb = consts.tile([P, DT, EW], BF16)
            nc.sync.dma_start(out=wk_sb[:], in_=wk_d.rearrange("(dt p) e -> p dt e", p=P))
            wqt_sb = consts.tile([P, ET, D], BF16)
            nc.sync.dma_start(out=wqt_sb[:], in_=wqt_d.rearrange("(et p) d -> p et d", p=P))
            wkt_sb = consts.tile([P, ET, D], BF16)
            nc.sync.dma_start(out=wkt_sb[:], in_=wkt_d.rearrange("(et p) d -> p et d", p=P))
            x_sb = consts.tile([P, NT, D], F32)
            nc.sync.dma_start(out=x_sb[:], in_=x_in.rearrange("(nt p) d -> p nt d", p=P))
            xi_sb = consts.tile([P, MT, D], BF16)
            nc.sync.dma_start(out=xi_sb[:], in_=xi_d.rearrange("(mt p) d -> p mt d", p=P))
            xit_sb = consts.tile([P, DT, ML], BF16)
            nc.sync.dma_start(out=xit_sb[:], in_=xit_d.rearrange("(dt p) m -> p dt m", p=P))

            from concourse.masks import make_identity

            ident_f = consts.tile([P, P], F32)
            make_identity(nc, ident_f[:])
            ident = consts.tile([P, P], F32R)
            nc.vector.tensor_copy(out=ident[:], in_=ident_f[:])
            ident_b = consts.tile([P, P], BF16)
            nc.vector.tensor_copy(out=ident_b[:], in_=ident_f[:])
            eps_t = consts.tile([P, 1], F32)
            nc.vector.memset(eps_t[:], EPS)

            F16 = mybir.dt.float16
            peer_prev = None
            for step in range(steps):
                pswa_ctx = tc.tile_pool(name="pswa", bufs=5, space="PSUM")
                pswA = pswa_ctx.__enter__()
                pswb_ctx = tc.tile_pool(name="pswb", bufs=3, space="PSUM")
                pswB = pswb_ctx.__enter__()

                # ======== x update (deferred from previous step's AllGather)
                # + LayerNorm forward, chunk-pipelined with the gT transposes
                # so the PE starts as soon as chunk 0 is ready ========
                xhatb = work.tile([P, NT, D], BF16, tag="xhatb")
                rstd = stats.tile([P, NT], F32, tag="rstd")
                gT = work.tile([P, DT, N], BF16, tag="gT")
                gtp = [
                    pswB.tile([P, 2, 512], BF16, tag="pswb", name=f"gtp{i}")
                    for i in range(ET)
                ]

                for nt in range(NT):
                    if peer_prev is not None:
                        for r in range(2):
                            nc.vector.scalar_tensor_tensor(
                                out=x_sb[:, nt, :], in0=peer_prev[:, r, nt, :], scalar=ALPHA,
                                in1=x_sb[:, nt, :], op0=OP.mult, op1=OP.add,
                            )
                    xt = x_sb[:, nt, :]
                    st = stats.tile([P, 3, 6], F32, tag="bnst")
                    xg = xt.rearrange("p (g s) -> p g s", s=256)
                    for gs in range(3):
                        nc.vector.bn_stats(out=st[:, gs, :], in_=xg[:, gs, :])
                    mv = stats.tile([P, 2], F32, tag="mv")
                    nc.vector.bn_aggr(out=mv[:], in_=st[:])
                    rr = rstd[:, nt : nt + 1]
                    nc.scalar.activation(out=rr, in_=mv[:, 1:2], func=AF.Sqrt, bias=eps_t[:], scale=1.0)
                    nc.vector.reciprocal(out=rr, in_=rr)
                    nmu = stats.tile([P, 1], F32, tag="nmu")
                    nc.vector.scalar_tensor_tensor(
                        out=nmu[:], in0=mv[:, 0:1], scalar=-1.0, in1=rr, op0=OP.mult, op1=OP.mult,
                    )
                    nc.vector.tensor_scalar(
                        out=xhatb[:, nt, :], in0=xt, scalar1=rr, scalar2=nmu[:],
                        op0=OP.mult, op1=OP.add,
                    )
                    # gT transposes for this chunk (columns nt of every dt)
                    for dt in range(DT):
                        nc.tensor.transpose(
                            gtp[dt // 2][:, dt % 2, nt * P : (nt + 1) * P],
                            xhatb[:, nt, dt * P : (dt + 1) * P], ident_b[:],
                        )
                for dp in range(ET):
                    nc.vector.tensor_copy(out=gT[:, 2 * dp : 2 * dp + 2, :], in_=gtp[dp][:])

                # ======== projections ========
                q = work.tile([P, NT, EW], BF16, tag="q")
                k = work.tile([P, NT, EW], BF16, tag="k")
                for nt in range(NT):
                    ppq = pswA.tile([P, 512], F32, tag="pswa")
                    ppk = pswA.tile([P, 512], F32, tag="pswa")
                    for dt in range(DT):
                        lh = gT[:, dt, nt * P : (nt + 1) * P]
                        nc.tensor.matmul(ppq[:, :EW], lh, wq_sb[:, dt, :], start=(dt == 0), stop=(dt == DT - 1))
                        nc.tensor.matmul(ppk[:, :EW], lh, wk_sb[:, dt, :], start=(dt == 0), stop=(dt == DT - 1))
                    nc.vector.tensor_copy(out=q[:, nt, :], in_=ppq[:, :EW])
                    nc.vector.tensor_copy(out=k[:, nt, :], in_=ppk[:, :EW])
                qT = work.tile([P, ET, N], BF16, tag="qT")
                kT = work.tile([P, ET, N], BF16, tag="kT")
                for dst, srct in ((qT, q), (kT, k)):
                    for et in range(ET):
                        pp = pswB.tile([P, 512], BF16, tag="pswb")
                        for nt in range(NT):
                            nc.tensor.transpose(
                                pp[:, nt * P : (nt + 1) * P],
                                srct[:, nt, et * P : (et + 1) * P], ident_b[:],
                            )
                        nc.vector.tensor_copy(out=dst[:, et, :], in_=pp[:])

                # ======== attention heads fused with Hopfield phase 1 ========
                # Per head: S/ST chunk matmuls + exps, then two Hopfield
                # h-chains (PE filler while the scalar engine runs the exps),
                # then dq/dk for the previous head (whose E/ET are done).
                dqTst = work.tile([P, ET, N], BF16, tag="dqTst")
                dkTst = work.tile([P, ET, N], BF16, tag="dkTst")

                hctx = {}
                rts = []

                def emit_sst(h, part):
                    et, eo = h // 2, (h % 2) * HD
                    if part == 0:
                        E = attp.tile([P, NT, N], BF16, tag="E")
                        ETt = attp.tile([P, NT, N], BF16, tag="ETt")
                        Z4 = attp.tile([P, NT], F32, tag="Z4")
                        Zi4 = attp.tile([P, NT], F32, tag="Zi4")
                        zrow = attp.tile([1, N], F32, tag="zrow")
                        ZinvB = attp.tile([HD, N], F32, tag="ZinvB")
                        qs = attp.tile([P, NT, HD], BF16, tag="qs")
                        # S = q k^T row chunks -> exp -> E (unnorm) + Z sums
                        for nt in range(NT):
                            ps = pswA.tile([P, 512], F32, tag="pswa")
                            nc.tensor.matmul(
                                ps[:], qT[eo : eo + HD, et, nt * P : (nt + 1) * P],
                                kT[eo : eo + HD, et, :], start=True, stop=True,
                            )
                            nc.scalar.activation(out=E[:, nt, :], in_=ps[:], func=AF.Exp)
                            nc.vector.tensor_reduce(
                                Z4[:, nt : nt + 1], E[:, nt, :], mybir.AxisListType.X, OP.add,
                            )
                        hctx[h] = (E, ETt, Z4, Zi4, zrow, ZinvB, qs)
                    else:
                        E, ETt, Z4, Zi4, zrow, ZinvB, qs = hctx[h]
                        # S^T = k q^T -> ET (unnormalized)
                        for jt in range(NT):
                            ps = pswA.tile([P, 512], F32, tag="pswa")
                            nc.tensor.matmul(
                                ps[:], kT[eo : eo + HD, et, jt * P : (jt + 1) * P],
                                qT[eo : eo + HD, et, :], start=True, stop=True,
                            )
                            nc.scalar.activation(out=ETt[:, jt, :], in_=ps[:], func=AF.Exp)

                def emit_hop_one(mt):
                    hp = pswB.tile([P, 512], F32, tag="pswb", name=f"hp{mt}")
                    for dt in range(DT):
                        nc.tensor.matmul(
                            hp[:], xit_sb[:, dt, mt * P : (mt + 1) * P], gT[:, dt, :],
                            start=(dt == 0), stop=(dt == DT - 1),
                        )
                    RT = rtp.tile([P, N], BF16, tag=f"RT{mt}")
                    nc.vector.tensor_scalar_max(out=RT[:], in0=hp[:], scalar1=0.0)
                    rts.append(RT)

                def emit_dqdk(h):
                    et, eo = h // 2, (h % 2) * HD
                    E, ETt, Z4, Zi4, zrow, ZinvB, qs = hctx.pop(h)
                    # Zinv column form (q scale) and broadcast row form (for
                    # the dqT free-dim scale); deferred one head so the tiny
                    # transposes never wait on the scalar exp backlog
                    nc.vector.reciprocal(out=Zi4[:], in_=Z4[:])
                    for nt in range(NT):
                        nc.vector.tensor_scalar_mul(
                            out=qs[:, nt, :], in0=q[:, nt, h * HD : (h + 1) * HD],
                            scalar1=Zi4[:, nt : nt + 1],
                        )
                    Zr = attp.tile([P, NT], F32R, tag="Zr")
                    nc.vector.tensor_copy(out=Zr[:], in_=Zi4[:])
                    ztp = pswB.tile([P, 512], F32, tag="pswb")
                    for c in range(NT):
                        nc.tensor.transpose(
                            ztp[:1, c * P : (c + 1) * P].bitcast(F32R), Zr[:, c : c + 1], ident[:],
                        )
                    nc.vector.tensor_copy(out=zrow[:1, :], in_=ztp[:1, :])
                    nc.gpsimd.partition_broadcast(ZinvB[:], zrow[:1, :], channels=HD)
                    # dkT_h = sum_i q'_ie E_ij
                    pk = pswA.tile([P, 512], F32, tag="pswa")
                    for nt in range(NT):
                        nc.tensor.matmul(
                            pk[:HD, :], qs[:, nt, :], E[:, nt, :],
                            start=(nt == 0), stop=(nt == NT - 1),
                        )
                    nc.vector.tensor_copy(out=dkTst[eo : eo + HD, et, :], in_=pk[:HD, :])
                    # dqT_h = (sum_j k_je ET_ji) * Zinv_i
                    pq = pswA.tile([P, 512], F32, tag="pswa")
                    for jt in range(NT):
                        nc.tensor.matmul(
                            pq[:HD, :], k[:, jt, h * HD : (h + 1) * HD], ETt[:, jt, :],
                            start=(jt == 0), stop=(jt == NT - 1),
                        )
                    nc.vector.tensor_tensor(
                        out=dqTst[eo : eo + HD, et, :], in0=pq[:HD, :], in1=ZinvB[:], op=OP.mult,
                    )

                for h in range(HL):
                    emit_sst(h, 0)
                    emit_hop_one(2 * h)
                    emit_sst(h, 1)
                    emit_hop_one(2 * h + 1)
                    if h > 0:
                        emit_dqdk(h - 1)

                emit_dqdk(HL - 1)
                pswb_ctx.__exit__(None, None, None)
                pswa_ctx.__exit__(None, None, None)

                # ======== phase 2: dg accumulation, untransposed [token, d] ========
                psdg_ctx = tc.tile_pool(name="psdg", bufs=1, space="PSUM")
                psdg = psdg_ctx.__enter__()
                dx = work.tile([P, NT, D], F32, tag="dx")
                dxb = work.tile([P, NT, D], BF16, tag="dxb")
                m1s = stats.tile([P, 2, NT], F32, tag="m1s")
                for nt in range(NT):
                    pds = [
                        psdg.tile([P, 512], F32, tag=f"pd{nt}{si}", name=f"pd{nt}{si}")
                        for si in range(len(DSEGS))
                    ]
                    for si, (dlo, dw) in enumerate(DSEGS):
                        for et in range(ET):
                            for d_t, w_t in ((dqTst, wqt_sb), (dkTst, wkt_sb)):
                                nc.tensor.matmul(
                                    pds[si][:, :dw], d_t[:, et, nt * P : (nt + 1) * P],
                                    w_t[:, et, dlo : dlo + dw],
                                    start=(et == 0 and d_t is dqTst), stop=False,
                                )
                    for si, (dlo, dw) in enumerate(DSEGS):
                        for mt in range(MT):
                            nc.tensor.matmul(
                                pds[si][:, :dw], rts[mt][:, nt * P : (nt + 1) * P],
                                xi_sb[:, mt, dlo : dlo + dw],
                                start=False, stop=(mt == MT - 1),
                            )
                    nc.vector.scalar_tensor_tensor(
                        out=dx[:, nt, 0:512], in0=pds[0][:], scalar=0.0, in1=xhatb[:, nt, 0:512],
                        op0=OP.bypass, op1=OP.bypass, accum_out=m1s[:, 0, nt : nt + 1],
                    )
                    nc.vector.scalar_tensor_tensor(
                        out=dx[:, nt, 512:768], in0=pds[1][:, :256], scalar=0.0, in1=xhatb[:, nt, 512:768],
                        op0=OP.bypass, op1=OP.bypass, accum_out=m1s[:, 1, nt : nt + 1],
                    )
                    # LayerNorm backward for this chunk (dx holds dg) -- kept
                    # inside the chunk loop so it overlaps later chunks' PE
                    # chains and feeds the first AllGather half early
                    rr = rstd[:, nt : nt + 1]
                    m1 = stats.tile([P, 1], F32, tag="m1")
                    nc.vector.tensor_tensor(out=m1[:], in0=m1s[:, 0, nt : nt + 1], in1=m1s[:, 1, nt : nt + 1], op=OP.add)
                    prodA = scr.tile([P, D], F32, tag="prodA")
                    u2 = stats.tile([P, 1], F32, tag="u2")
                    nc.vector.scalar_tensor_tensor(
                        out=prodA[:], in0=dx[:, nt, :], scalar=1.0, in1=xhatb[:, nt, :],
                        op0=OP.mult, op1=OP.mult, accum_out=u2[:],
                    )
                    c1 = stats.tile([P, 1], F32, tag="c1")
                    nc.vector.scalar_tensor_tensor(
                        out=c1[:], in0=m1[:], scalar=1.0 / D, in1=rr, op0=OP.mult, op1=OP.mult,
                    )
                    c2 = stats.tile([P, 1], F32, tag="c2")
                    nc.vector.scalar_tensor_tensor(
                        out=c2[:], in0=u2[:], scalar=-1.0 / D, in1=rr, op0=OP.mult, op1=OP.mult,
                    )
                    lnv = scr.tile([P, D], F32, tag="lnv")
                    nc.vector.tensor_scalar(
                        out=lnv[:], in0=dx[:, nt, :], scalar1=rr, scalar2=c1[:],
                        op0=OP.mult, op1=OP.subtract,
                    )
                    nc.vector.scalar_tensor_tensor(
                        out=dxb[:, nt, :], in0=xhatb[:, nt, :], scalar=c2[:], in1=lnv[:],
                        op0=OP.mult, op1=OP.add,
                    )
                psdg_ctx.__exit__(None, None, None)

                # ======== pair exchange (AllGather: no slow CC-core reduce;
                # the pair sum is folded into the deferred x update).  Split
                # in two halves: the first overlaps the second half of the
                # dg accumulation / LayerNorm-backward. ========
                if with_ar:
                    peer = work.tile([P, 2, NT, D], BF16, tag="peer")
                    HN = N // 2
                    arouts = []
                    for g in range(2):
                        arin = drp.tile([HN, D], BF16, tag=f"arin{g}", name=f"arin{g}")
                        arout = drp.tile([2 * HN, D], BF16, tag=f"arout{g}", name=f"arout{g}")
                        for c in range(2):
                            nt = 2 * g + c
                            nc.sync.dma_start(out=arin[c * P : (c + 1) * P, :], in_=dxb[:, nt, :])
                        nc.gpsimd.collective_compute(
                            "AllGather", OP.bypass, replica_groups=REPLICA_GROUPS,
                            ins=[arin.opt()], outs=[arout.opt()],
                        )
                        arouts.append(arout)
                    # peer readbacks AFTER both collectives are queued, so the
                    # second collective's inputs are not stuck behind them
                    for g in range(2):
                        for r in range(2):
                            for c in range(2):
                                nt = 2 * g + c
                                nc.sync.dma_start(
                                    out=peer[:, r, nt, :],
                                    in_=arouts[g][r * HN + c * P : r * HN + (c + 1) * P, :],
                                )
                    peer_prev = peer
                else:
                    for nt in range(NT):
                        nc.vector.scalar_tensor_tensor(
                            out=x_sb[:, nt, :], in0=dxb[:, nt, :], scalar=ALPHA,
                            in1=x_sb[:, nt, :], op0=OP.mult, op1=OP.add,
                        )

            if peer_prev is not None:
                for nt in range(NT):
                    for r in range(2):
                        nc.vector.scalar_tensor_tensor(
                            out=x_sb[:, nt, :], in0=peer_prev[:, r, nt, :], scalar=ALPHA,
                            in1=x_sb[:, nt, :], op0=OP.mult, op1=OP.add,
                        )
            for nt in range(NT):
                nc.sync.dma_start(out=x_out[nt * P : (nt + 1) * P, :], in_=x_sb[:, nt, :])

    nc.compile()
    return nc


def _prep_inputs(x, gamma, delta, Wq, Wk, xi):
    """Build the 8 per-core input dicts (host-side sharding + weight folding)."""
    assert np.allclose(delta, 0.0), "kernel requires delta == 0"
    beta_sqrt = np.float32(1.0 / np.sqrt(np.sqrt(np.float32(HD))))
    # sqrt(beta) = (1/sqrt(HD))^(1/2) = HD^(-1/4)
    g = gamma.astype(np.float32)
    in_maps = []
    for c in range(8):
        b, j = c // 2, c % 2
        hs = slice(j * HL, (j + 1) * HL)
        wq_l = (Wq[hs] * g[None, :, None]).transpose(1, 0, 2).reshape(D, EW)
        wk_l = (Wk[hs] * g[None, :, None]).transpose(1, 0, 2).reshape(D, EW)
        wqt_l = (Wq[hs] * g[None, :, None]).transpose(0, 2, 1).reshape(EW, D)
        wkt_l = (Wk[hs] * g[None, :, None]).transpose(0, 2, 1).reshape(EW, D)
        xi_l = xi[j * ML : (j + 1) * ML] * g[None, :]
        import ml_dtypes

        bf = ml_dtypes.bfloat16
        in_maps.append(
            {
                "x": np.ascontiguousarray(x[b]),
                "wq": np.ascontiguousarray(wq_l * beta_sqrt).astype(bf),
                "wk": np.ascontiguousarray(wk_l * beta_sqrt).astype(bf),
                "wqt": np.ascontiguousarray(wqt_l / beta_sqrt).astype(bf),
                "wkt": np.ascontiguousarray(wkt_l / beta_sqrt).astype(bf),
                "xi": np.ascontiguousarray(xi_l).astype(bf),
                "xit": np.ascontiguousarray(xi_l.T).astype(bf),
            }
        )
    return in_maps


_NC_CACHE = {}


def _get_nc(steps=STEPS, with_ar=True):
    key = (steps, with_ar)
    if key not in _NC_CACHE:
        _NC_CACHE[key] = build_kernel(steps, with_ar)
    return _NC_CACHE[key]


def kernel(x, gamma, delta, Wq, Wk, xi):
    from concourse.bass_utils import run_bass_kernel_spmd

    x = np.asarray(x, dtype=np.float32)
    in_maps = _prep_inputs(
        x,
        np.asarray(gamma, np.float32),
        np.asarray(delta, np.float32),
        np.asarray(Wq, np.float32),
        np.asarray(Wk, np.float32),
        np.asarray(xi, np.float32),
    )
    nc = _get_nc()
    res = run_bass_kernel_spmd(nc, in_maps, list(range(8)))
    out = np.stack([res.results[2 * b]["x_out"] for b in range(B)], axis=0)
    return out.astype(np.float32)
